# revision 18
# baseline (speedup 1.0000x reference)
"""BQuantConv1d Trainium2 kernel.

Math: the reference's per-token LUT + gather is algebraically a matmul:
  out[n, f] = sum_i x[n, i] * W[i, f] + bias[f]
  W[8g+j, f] = sum_b scale[b, f] * (2*bit_{7-j}(binary[b, g, f]) - 1)

Sharding: 2 token-groups x 4 f-groups over 8 cores, no collectives
(host slices inputs / concatenates outputs; layout-only host work).
Contraction order is permuted to i' = j*128 + g (host permutes xT rows to
match) so each decoded weight chunk j lands on contiguous partitions.

Per core:
  - decode W'(1024, 256) from int16 codes with a sign-bit trick:
    W element = +-scale[b, f] exactly, built by XORing the fp16 scale's
    sign bit (scales arrive sign-pre-flipped) with the masked quant bit
    (c << (8+j)) & 0x8000, as int32 SWAR on DVE (bitvec ops are DVE-only
    and 32-bit-only on walrus); the 8-way b-reduction is an fp16 add
    tree with the first level (h1) on DVE and the h2/w levels offloaded
    to the otherwise-idle GPSIMD engine.  Chunks 0/1 run TS/xor in
    b-halves gated on half-sized cd/sc DMAs; chunk 7's whole tree runs
    on DVE split by f-half at every level so the fb=0 matmuls start
    while fb=1 is still reducing.
  - outT[f_shard, n_shard] = W'.T @ xT on the PE in fp16, accumulating
    the 8 contraction chunks across 8 concurrent PSUM banks (f32).
  - The PE p-state ramp (cost model: ~7us of continuous execution
    before full clock) is paid with low-priority zero-matmul fillers
    that keep the PE busy from ~1us instead of bias-seed matmuls; the
    filler bank's real accumulation chain is dep-deferred behind them
    and doubles as tail work.  The bias rides the PSUM evacuation for
    free (DVE tensor_scalar add with a per-partition fp32 bias column /
    ACT Identity activation with an AP bias operand).
  - PSUM pairs evacuated as fp16 (copies alternating DVE/ACT) into
    double-wide tiles; each ch-pair ships as ONE partition-major DMA,
    fb0 pairs on the gpsimd SWDGE queue, fb1 pairs on SP/HWDGE, so the
    two issue pipes overlap in the tail.
"""

import numpy as np

try:
    import concourse.bass as bass  # noqa: F401
except ImportError:
    import sys

    sys.path.insert(0, "/opt/trn_rl_repo")
    import concourse.bass as bass  # noqa: F401

import concourse.bacc as bacc
import concourse.mybir as mybir
import concourse.tile as tile

B, T, NX, NF = 2, 2048, 1024, 1024
N_TOK = B * T
BITS = 8
G = NX // 8  # 128 code groups
PT, PF = 2, 4  # token-parallel x feature-parallel
TOK = N_TOK // PT  # tokens per core
NFS = NF // PF  # output features per core
P = 128
MM_N = 512  # moving free dim per matmul
N_FILL = 24  # PE warm-up/pacing zero-matmuls (tuned against TimelineSim)

AX = mybir.AxisListType
OP = mybir.AluOpType
F32 = mybir.dt.float32
BF16 = mybir.dt.float16  # compute dtype (fp16: same SWAR, more mantissa)
I16 = mybir.dt.int16
I32 = mybir.dt.int32
ACT_F = mybir.ActivationFunctionType
BF16NP = np.float16

MSK = -2147450880  # 0x80008000 as int32


def build_graph(nc, tok=TOK, nfs=NFS):
    nfb = nfs // P  # f blocks of 128 (2)
    nch = tok // MM_N  # moving chunks (4)
    xt_d = nc.dram_tensor("xt", (8, P, tok), BF16, kind="ExternalInput")
    cd_d = nc.dram_tensor("codes", (P, 8 * nfs), I16, kind="ExternalInput")
    sc_d = nc.dram_tensor("scales", (P, 8 * nfs), BF16, kind="ExternalInput")
    bi_d = nc.dram_tensor("biasc", (P, nfb), F32, kind="ExternalInput")
    out_d = nc.dram_tensor("out", (nfb, P, tok), BF16, kind="ExternalOutput")
    # fb0 banks host the warm-up fillers; their real chains get dep-deferred
    # behind their fillers and become gap-fill work mid-stream
    FILL_BANKS = [(0, ch) for ch in range(nch)]

    with tile.TileContext(nc) as tc:
        with (
            tc.tile_pool(name="xp", bufs=8) as xp,
            tc.tile_pool(name="cp", bufs=8) as cp,
            tc.tile_pool(name="wp", bufs=8) as wp,
            tc.tile_pool(name="qp", bufs=6) as qp,
            tc.tile_pool(name="cst", bufs=1) as cst,
            tc.tile_pool(name="op", bufs=8) as op_,
            tc.tile_pool(name="pp", bufs=8, space="PSUM") as pp,
        ):
            # --- loads; codes/scales first, halves interleaved: chunk 0/1's
            # decode runs in b-halves gated on each half-DMA ---
            H = 4 * nfs
            cd = cp.tile([P, 8 * nfs], I16, tag="cd")
            sc_bc = cst.tile([P, 8 * nfs], BF16, tag="sc_bc")
            nc.sync.dma_start(cd[:, :H], cd_d[:, :H])
            nc.sync.dma_start(sc_bc[:, :H], sc_d[:, :H])
            nc.sync.dma_start(cd[:, H:], cd_d[:, H:])
            nc.sync.dma_start(sc_bc[:, H:], sc_d[:, H:])
            biasc = cst.tile([P, nfb], F32, tag="biasc")
            nc.sync.dma_start(biasc[:], bi_d[:])
            zero_m = cst.tile([P, MM_N], BF16, tag="zero_m")
            nc.vector.memset(zero_m[:], 0.0)  # DVE: idle until cd lands
            xts = []
            for j in range(8):
                xt = xp.tile([P, tok], BF16, tag="xt")
                nc.sync.dma_start(xt[:], xt_d[j])
                xts.append(xt)

            # PSUM banks.  FILLB is reset by the first zero-filler; every
            # other bank starts accumulation at its j=0 matmul.
            pss = {}
            for fb in range(nfb):
                for ch in range(nch):
                    ps = pp.tile([P, MM_N], F32, tag="ps", name=f"ps{fb}_{ch}")
                    pss[(fb, ch)] = ps

            # --- PE warm-up fillers: zero-weight matmuls spread across the
            # fb0 banks.  Low priority: the scheduler runs them only when
            # no real matmul is ready; they pay the cost model's ~10us PE
            # p-state ramp starting at ~0.9us, while the PE would
            # otherwise idle waiting for the first decoded W chunk.
            with tc.high_priority(offset=-(10**6)):
                for i in range(N_FILL):
                    bank = FILL_BANKS[i % len(FILL_BANKS)]
                    nc.tensor.matmul(
                        pss[bank][:], zero_m[:, :P], zero_m[:],
                        start=(i < len(FILL_BANKS)), stop=False,
                    )

            # --- decode W chunks ---
            # Sign-bit trick: masked quant bit (inverted) XORed onto the
            # fp16 scale's sign gives +-scale exactly.  Bitvec ops are
            # DVE-only and 32-bit-only on walrus, so they run as int32 SWAR
            # over int16-lane pairs: a left shift by 8+j sources each
            # lane's bit 15 from within the same lane, and the 0x80008000
            # mask keeps only the two sign bits.  The bit inversion is
            # folded into a one-time sign-flip of the scale tile:
            #   ((c << (8+j)) & M) ^ (sc ^ M)  ==  ((~c << (8+j)) & M) ^ sc
            def emit_ts(j):
                sg = qp.tile([P, 8 * nfs], I16, tag="sg", name=f"sg{j}")
                nc.vector.tensor_scalar(
                    sg[:].bitcast(I32), cd[:].bitcast(I32), 8 + j, MSK,
                    OP.logical_shift_left, OP.bitwise_and,
                )
                return sg

            def emit_xor(j, sg):
                wsg = qp.tile([P, 8 * nfs], I16, tag="wsg", name=f"wsg{j}")
                nc.vector.tensor_tensor(
                    wsg[:].bitcast(I32), sg[:].bitcast(I32),
                    sc_bc[:].bitcast(I32), OP.bitwise_xor,
                )
                return wsg

            def emit_h1(j, wsg):
                wv = wsg[:].bitcast(BF16)
                h1 = qp.tile([P, 4 * nfs], BF16, tag="h1", name=f"h1_{j}")
                nc.vector.tensor_tensor(
                    h1[:], wv[:, : 4 * nfs], wv[:, 4 * nfs :], OP.add
                )
                return h1

            def tree_tail(j, h1, teng):
                h2 = qp.tile([P, 2 * nfs], BF16, tag="h2", name=f"h2_{j}")
                teng.tensor_tensor(
                    h2[:], h1[:, : 2 * nfs], h1[:, 2 * nfs :], OP.add
                )
                w = wp.tile([P, nfs], BF16, tag="w", name=f"w{j}")
                teng.tensor_tensor(w[:], h2[:, :nfs], h2[:, nfs:], OP.add)
                return w

            # b-reduction fp16 add tree: h1 on DVE; h2/w on GPSIMD for
            # chunks 0-6.  The next chunk's TS is emitted BETWEEN xor_j and
            # h1_j: it is always ready, so it fills the write-ack window
            # after xor_j and the scheduler then runs h1_j instead of
            # parking the next 1.1us xor in front of it.
            ws = {}
            # chunks 0/1 in b-halves so each half starts on its half-DMA
            # (fills the DVE while the second cd/sc halves are in flight)
            sgs, wsgs = {}, {}
            for j in (0, 1):
                sg = qp.tile([P, 8 * nfs], I16, tag="sg", name=f"sg{j}")
                wsg = qp.tile([P, 8 * nfs], I16, tag="wsg", name=f"wsg{j}")
                for half in range(2):
                    lo, hi = half * H, (half + 1) * H
                    nc.vector.tensor_scalar(
                        sg[:, lo:hi].bitcast(I32), cd[:, lo:hi].bitcast(I32),
                        8 + j, MSK, OP.logical_shift_left, OP.bitwise_and,
                    )
                    nc.vector.tensor_tensor(
                        wsg[:, lo:hi].bitcast(I32),
                        sg[:, lo:hi].bitcast(I32),
                        sc_bc[:, lo:hi].bitcast(I32), OP.bitwise_xor,
                    )
                sgs[j], wsgs[j] = sg, wsg
            for j in range(1, 8):
                if j > 1:
                    sgs[j] = emit_ts(j)
                h1 = emit_h1(j - 1, wsgs[j - 1])
                # chunk 6's whole tree stays on DVE: its W must land well
                # before W7 or the per-bank j6->j7 accumulation chains
                # serialize 16 matmuls into the tail
                teng = nc.vector if j - 1 == 6 else nc.gpsimd
                ws[j - 1] = tree_tail(j - 1, h1, teng)
                if j > 1:
                    wsgs[j] = emit_xor(j, sgs[j])
            # chunk 7: all three tree levels split by f-half on DVE so
            # W7[fb0] lands ~0.7us after xor7 and the PE tail starts early
            wv7 = wsgs[7][:].bitcast(BF16).rearrange("p (b f) -> p b f", b=8)
            w7 = wp.tile([P, nfs], BF16, tag="w", name="w7")
            for half in range(2):
                f0, f1 = half * P, (half + 1) * P
                h1h = qp.tile([P, 4, P], BF16, tag="h1h", name=f"h1h{half}")
                nc.vector.tensor_tensor(
                    h1h[:], wv7[:, :4, f0:f1], wv7[:, 4:, f0:f1], OP.add
                )
                h2h = qp.tile([P, 2, P], BF16, tag="h2h", name=f"h2h{half}")
                nc.vector.tensor_tensor(
                    h2h[:], h1h[:, :2], h1h[:, 2:], OP.add
                )
                nc.vector.tensor_tensor(
                    w7[:, f0:f1], h2h[:, 0], h2h[:, 1], OP.add
                )
            ws[7] = w7

            # --- matmul: outT[f, n] = sum_j W_j.T @ xT_j (+bias at evac) ---
            # j outermost: each W chunk feeds the PE as soon as it is
            # decoded, all nfb*nch PSUM banks accumulate concurrently.
            # The last chunk is issued bank-by-bank so evacuation and
            # output DMA overlap the remaining j=7 matmuls.
            for j in range(7):
                for fb in range(nfb):
                    for ch in range(nch):
                        nc.tensor.matmul(
                            pss[(fb, ch)][:],
                            ws[j][:, fb * P : (fb + 1) * P],
                            xts[j][:, ch * MM_N : (ch + 1) * MM_N],
                            start=(j == 0 and (fb, ch) not in FILL_BANKS),
                            stop=False,
                        )
            # evacuation alternates DVE/ACT (GPSIMD cannot read PSUM) and
            # folds the bias in for free: DVE as tensor_scalar add with a
            # per-partition fp32 bias column, ACT as Identity with the AP
            # bias operand.  Banks pair into double-wide tiles; each pair
            # ships as ONE DMA, fb0 pairs on the gpsimd SWDGE queue and
            # fb1 pairs on SP/HWDGE so the issue pipes overlap.
            # first three bank-pairs ship as pair DMAs (gp/SP/gp), the last
            # two banks as singles on SP so the final transfer is short
            plan = [((0, 0), (0, 1), nc.gpsimd), ((0, 2), (0, 3), nc.sync),
                    ((1, 0), (1, 1), nc.gpsimd), ((1, 2), None, nc.sync),
                    ((1, 3), None, nc.sync)]
            k = 0
            evac_tiles = {}
            for banks in plan:
                b0, b1, deng = banks
                width = 2 * MM_N if b1 is not None else MM_N
                obw = op_.tile([P, width], BF16, tag="obw", name=f"obw{k}")
                for half, bk in enumerate((b0, b1)):
                    if bk is None:
                        continue
                    fb, ch = bk
                    bcol = biasc[:, fb : fb + 1]
                    nc.tensor.matmul(
                        pss[bk][:],
                        ws[7][:, fb * P : (fb + 1) * P],
                        xts[7][:, ch * MM_N : (ch + 1) * MM_N],
                        start=False,
                        stop=True,
                    )
                    dst = obw[:, half * MM_N : (half + 1) * MM_N]
                    if k % 2 == 0:
                        nc.vector.tensor_scalar(
                            dst, pss[bk][:], bcol, None, OP.add
                        )
                    else:
                        nc.scalar.activation(
                            dst, pss[bk][:], ACT_F.Identity, bias=bcol,
                        )
                    k += 1
                fb0_, ch0_ = b0
                deng.dma_start(
                    out_d[fb0_][:, ch0_ * MM_N : ch0_ * MM_N + width],
                    obw[:],
                )
    nc.compile()
    return nc


_I_PERM = 8 * (np.arange(NX) % G) + np.arange(NX) // G  # i' -> i


def host_prep(x, binary, scale, bias):
    """Layout-only sharding (plus x's fp16 compute-precision cast).
    Returns in_maps for cores 0..7 (pt = c//PF, pf = c%PF)."""
    x2 = np.ascontiguousarray(x.reshape(N_TOK, NX).T)[_I_PERM]  # (NX, N)
    x2 = x2.astype(BF16NP)  # compute dtype
    binary16 = binary.astype(np.int16)  # lossless: codes are 0..255
    in_maps = []
    for c in range(8):
        pt, pf = c // PF, c % PF
        f0 = pf * NFS
        xs = np.ascontiguousarray(x2[:, pt * TOK : (pt + 1) * TOK]).reshape(
            8, P, TOK
        )
        cs = np.ascontiguousarray(
            binary16[:, :, f0 : f0 + NFS].transpose(1, 0, 2)
        ).reshape(P, 8 * NFS)
        ss = np.ascontiguousarray(
            np.broadcast_to(
                (-scale[:, f0 : f0 + NFS].astype(BF16NP)).reshape(1, 8 * NFS),
                (P, 8 * NFS),
            )
        )
        bs = np.ascontiguousarray(
            bias[f0 : f0 + NFS].astype(np.float32).reshape(NFS // P, P).T
        )
        in_maps.append({"xt": xs, "codes": cs, "scales": ss, "biasc": bs})
    return in_maps


def host_assemble(results):
    """results[c]["out"]: (NFB, 128, TOK) -> full (B, T, NF)."""
    outT = np.empty((NF, N_TOK), dtype=np.float32)
    for c in range(8):
        pt, pf = c // PF, c % PF
        o = np.asarray(results[c]["out"], dtype=np.float32).reshape(NFS, TOK)
        outT[pf * NFS : (pf + 1) * NFS, pt * TOK : (pt + 1) * TOK] = o
    return np.ascontiguousarray(outT.T).reshape(B, T, NF)


_NC_CACHE = {}


def _get_nc():
    if "nc" not in _NC_CACHE:
        nc = bacc.Bacc(None, target_bir_lowering=False)
        build_graph(nc)
        _NC_CACHE["nc"] = nc
    return _NC_CACHE["nc"]


def kernel(**inputs):
    from concourse.bass_utils import run_bass_kernel_spmd

    inputs = {k: np.asarray(v) for k, v in inputs.items()}
    in_maps = host_prep(
        inputs["x"], inputs["binary"], inputs["scale"], inputs["bias"]
    )
    res = run_bass_kernel_spmd(_get_nc(), in_maps, core_ids=list(range(8)))
    return host_assemble(res.results)


# revision 19
# speedup vs baseline: 1.0184x; 1.0184x over previous
"""BQuantConv1d Trainium2 kernel.

Math: the reference's per-token LUT + gather is algebraically a matmul:
  out[n, f] = sum_i x[n, i] * W[i, f] + bias[f]
  W[8g+j, f] = sum_b scale[b, f] * (2*bit_{7-j}(binary[b, g, f]) - 1)

Sharding: 2 token-groups x 4 f-groups over 8 cores, no collectives
(host slices inputs / concatenates outputs; layout-only host work).
Contraction order is permuted to i' = j*128 + g (host permutes xT rows to
match) so each decoded weight chunk j lands on contiguous partitions.

Per core:
  - decode W'(1024, 256) from int16 codes with a sign-bit trick:
    W element = +-scale[b, f] exactly, built by XORing the fp16 scale's
    sign bit (scales arrive sign-pre-flipped) with the masked quant bit
    (c << (8+j)) & 0x8000, as int32 SWAR on DVE (bitvec ops are DVE-only
    and 32-bit-only on walrus); the 8-way b-reduction is an fp16 add
    tree with the first level (h1) on DVE and the h2/w levels offloaded
    to the otherwise-idle GPSIMD engine.  Chunks 0/1 run TS/xor in
    b-halves gated on half-sized cd/sc DMAs; chunk 7's whole tree runs
    on DVE split by f-half at every level so the fb=0 matmuls start
    while fb=1 is still reducing.
  - outT[f_shard, n_shard] = W'.T @ xT on the PE in fp16, accumulating
    the 8 contraction chunks across 8 concurrent PSUM banks (f32).
  - The PE p-state ramp (cost model: ~7us of continuous execution
    before full clock) is paid with low-priority zero-matmul fillers
    that keep the PE busy from ~1us instead of bias-seed matmuls; the
    filler bank's real accumulation chain is dep-deferred behind them
    and doubles as tail work.  The bias rides the PSUM evacuation for
    free (DVE tensor_scalar add with a per-partition fp32 bias column /
    ACT Identity activation with an AP bias operand).
  - PSUM pairs evacuated as fp16 (copies alternating DVE/ACT) into
    double-wide tiles; each ch-pair ships as ONE partition-major DMA,
    fb0 pairs on the gpsimd SWDGE queue, fb1 pairs on SP/HWDGE, so the
    two issue pipes overlap in the tail.
"""

import numpy as np

try:
    import concourse.bass as bass  # noqa: F401
except ImportError:
    import sys

    sys.path.insert(0, "/opt/trn_rl_repo")
    import concourse.bass as bass  # noqa: F401

import concourse.bacc as bacc
import concourse.mybir as mybir
import concourse.tile as tile

B, T, NX, NF = 2, 2048, 1024, 1024
N_TOK = B * T
BITS = 8
G = NX // 8  # 128 code groups
PT, PF = 2, 4  # token-parallel x feature-parallel
TOK = N_TOK // PT  # tokens per core
NFS = NF // PF  # output features per core
P = 128
MM_N = 512  # moving free dim per matmul
N_FILL = 24  # PE warm-up/pacing zero-matmuls (tuned against TimelineSim)

AX = mybir.AxisListType
OP = mybir.AluOpType
F32 = mybir.dt.float32
BF16 = mybir.dt.float16  # compute dtype (fp16: same SWAR, more mantissa)
I16 = mybir.dt.int16
I32 = mybir.dt.int32
ACT_F = mybir.ActivationFunctionType
BF16NP = np.float16

MSK = -2147450880  # 0x80008000 as int32


def build_graph(nc, tok=TOK, nfs=NFS):
    nfb = nfs // P  # f blocks of 128 (2)
    nch = tok // MM_N  # moving chunks (4)
    xt_d = nc.dram_tensor("xt", (8, P, tok), BF16, kind="ExternalInput")
    cd_d = nc.dram_tensor("codes", (P, 8 * nfs), I16, kind="ExternalInput")
    sc_d = nc.dram_tensor("scales", (P, 8 * nfs), BF16, kind="ExternalInput")
    bi_d = nc.dram_tensor("biasc", (P, nfb), F32, kind="ExternalInput")
    out_d = nc.dram_tensor("out", (nfb, P, tok), BF16, kind="ExternalOutput")
    # fb0 banks host the warm-up fillers; their real chains get dep-deferred
    # behind their fillers and become gap-fill work mid-stream
    FILL_BANKS = [(0, ch) for ch in range(nch)]

    with tile.TileContext(nc) as tc:
        with (
            tc.tile_pool(name="xp", bufs=8) as xp,
            tc.tile_pool(name="cp", bufs=8) as cp,
            tc.tile_pool(name="wp", bufs=8) as wp,
            tc.tile_pool(name="qp", bufs=6) as qp,
            tc.tile_pool(name="cst", bufs=1) as cst,
            tc.tile_pool(name="op", bufs=8) as op_,
            tc.tile_pool(name="pp", bufs=8, space="PSUM") as pp,
        ):
            # --- loads; codes/scales first, halves interleaved: chunk 0/1's
            # decode runs in b-halves gated on each half-DMA ---
            H = 4 * nfs
            cd = cp.tile([P, 8 * nfs], I16, tag="cd")
            sc_bc = cst.tile([P, 8 * nfs], BF16, tag="sc_bc")
            nc.sync.dma_start(cd[:, :H], cd_d[:, :H])
            nc.sync.dma_start(sc_bc[:, :H], sc_d[:, :H])
            nc.sync.dma_start(cd[:, H:], cd_d[:, H:])
            nc.sync.dma_start(sc_bc[:, H:], sc_d[:, H:])
            biasc = cst.tile([P, nfb], F32, tag="biasc")
            nc.sync.dma_start(biasc[:], bi_d[:])
            zero_m = cst.tile([P, MM_N], BF16, tag="zero_m")
            nc.vector.memset(zero_m[:], 0.0)  # DVE: idle until cd lands
            xts = []
            for j in range(8):
                xt = xp.tile([P, tok], BF16, tag="xt")
                nc.sync.dma_start(xt[:], xt_d[j])
                xts.append(xt)

            # PSUM banks.  FILLB is reset by the first zero-filler; every
            # other bank starts accumulation at its j=0 matmul.
            pss = {}
            for fb in range(nfb):
                for ch in range(nch):
                    ps = pp.tile([P, MM_N], F32, tag="ps", name=f"ps{fb}_{ch}")
                    pss[(fb, ch)] = ps

            # --- PE warm-up fillers: zero-weight matmuls spread across the
            # fb0 banks.  Low priority: the scheduler runs them only when
            # no real matmul is ready; they pay the cost model's ~10us PE
            # p-state ramp starting at ~0.9us, while the PE would
            # otherwise idle waiting for the first decoded W chunk.
            with tc.high_priority(offset=-(10**6)):
                for i in range(N_FILL):
                    bank = FILL_BANKS[i % len(FILL_BANKS)]
                    nc.tensor.matmul(
                        pss[bank][:], zero_m[:, :P], zero_m[:],
                        start=(i < len(FILL_BANKS)), stop=False,
                    )

            # --- decode W chunks ---
            # Sign-bit trick: masked quant bit (inverted) XORed onto the
            # fp16 scale's sign gives +-scale exactly.  Bitvec ops are
            # DVE-only and 32-bit-only on walrus, so they run as int32 SWAR
            # over int16-lane pairs: a left shift by 8+j sources each
            # lane's bit 15 from within the same lane, and the 0x80008000
            # mask keeps only the two sign bits.  The bit inversion is
            # folded into a one-time sign-flip of the scale tile:
            #   ((c << (8+j)) & M) ^ (sc ^ M)  ==  ((~c << (8+j)) & M) ^ sc
            def emit_ts(j):
                sg = qp.tile([P, 8 * nfs], I16, tag="sg", name=f"sg{j}")
                nc.vector.tensor_scalar(
                    sg[:].bitcast(I32), cd[:].bitcast(I32), 8 + j, MSK,
                    OP.logical_shift_left, OP.bitwise_and,
                )
                return sg

            def emit_xor(j, sg):
                wsg = qp.tile([P, 8 * nfs], I16, tag="wsg", name=f"wsg{j}")
                nc.vector.tensor_tensor(
                    wsg[:].bitcast(I32), sg[:].bitcast(I32),
                    sc_bc[:].bitcast(I32), OP.bitwise_xor,
                )
                return wsg

            def emit_h1(j, wsg):
                wv = wsg[:].bitcast(BF16)
                h1 = qp.tile([P, 4 * nfs], BF16, tag="h1", name=f"h1_{j}")
                nc.vector.tensor_tensor(
                    h1[:], wv[:, : 4 * nfs], wv[:, 4 * nfs :], OP.add
                )
                return h1

            def tree_tail(j, h1, teng):
                h2 = qp.tile([P, 2 * nfs], BF16, tag="h2", name=f"h2_{j}")
                teng.tensor_tensor(
                    h2[:], h1[:, : 2 * nfs], h1[:, 2 * nfs :], OP.add
                )
                w = wp.tile([P, nfs], BF16, tag="w", name=f"w{j}")
                teng.tensor_tensor(w[:], h2[:, :nfs], h2[:, nfs:], OP.add)
                return w

            # b-reduction fp16 add tree: h1 on DVE; h2/w on GPSIMD for
            # chunks 0-6.  The next chunk's TS is emitted BETWEEN xor_j and
            # h1_j: it is always ready, so it fills the write-ack window
            # after xor_j and the scheduler then runs h1_j instead of
            # parking the next 1.1us xor in front of it.
            ws = {}
            # chunks 0/1 in b-halves so each half starts on its half-DMA
            # (fills the DVE while the second cd/sc halves are in flight)
            sgs, wsgs = {}, {}
            for j in (0, 1):
                sg = qp.tile([P, 8 * nfs], I16, tag="sg", name=f"sg{j}")
                wsg = qp.tile([P, 8 * nfs], I16, tag="wsg", name=f"wsg{j}")
                for half in range(2):
                    lo, hi = half * H, (half + 1) * H
                    nc.vector.tensor_scalar(
                        sg[:, lo:hi].bitcast(I32), cd[:, lo:hi].bitcast(I32),
                        8 + j, MSK, OP.logical_shift_left, OP.bitwise_and,
                    )
                    nc.vector.tensor_tensor(
                        wsg[:, lo:hi].bitcast(I32),
                        sg[:, lo:hi].bitcast(I32),
                        sc_bc[:, lo:hi].bitcast(I32), OP.bitwise_xor,
                    )
                sgs[j], wsgs[j] = sg, wsg
            for j in range(1, 7):
                if j > 1:
                    sgs[j] = emit_ts(j)
                h1 = emit_h1(j - 1, wsgs[j - 1])
                ws[j - 1] = tree_tail(j - 1, h1, nc.gpsimd)
                if j > 1:
                    wsgs[j] = emit_xor(j, sgs[j])

            # --- endgame: chunks 6 and 7 reduce on DVE, f-half-split at
            # every tree level.  Chunk 6's W must land ~2us before chunk
            # 7's: each PSUM bank's j6->j7 accumulation is order-chained,
            # so a late W6 serializes 16 matmuls into the tail.  TS7 slots
            # between tree levels to fill the same-engine ack windows. ---
            def split_tree(j, wsg, w_tile, pieces):
                wv = wsg[:].bitcast(BF16).rearrange("p (b f) -> p b f", b=8)
                for half in range(2):
                    f0, f1 = half * P, (half + 1) * P
                    h1h = qp.tile([P, 4, P], BF16, tag="h1h",
                                  name=f"h1h{j}_{half}")
                    nc.vector.tensor_tensor(
                        h1h[:], wv[:, :4, f0:f1], wv[:, 4:, f0:f1], OP.add
                    )
                    h2h = qp.tile([P, 2, P], BF16, tag="h2h",
                                  name=f"h2h{j}_{half}")
                    nc.vector.tensor_tensor(h2h[:], h1h[:, :2], h1h[:, 2:],
                                            OP.add)
                    nc.vector.tensor_tensor(
                        w_tile[:, f0:f1], h2h[:, 0], h2h[:, 1], OP.add
                    )
                    if half == 0 and pieces:
                        pieces.pop()()

            w6 = wp.tile([P, nfs], BF16, tag="w", name="w6")
            w7 = wp.tile([P, nfs], BF16, tag="w", name="w7")
            split_tree(6, wsgs[6], w6, [lambda: sgs.__setitem__(7, emit_ts(7))])
            ws[6] = w6
            wsgs[7] = emit_xor(7, sgs[7])
            split_tree(7, wsgs[7], w7, [])
            ws[7] = w7

            # --- matmul: outT[f, n] = sum_j W_j.T @ xT_j (+bias at evac) ---
            # j outermost: each W chunk feeds the PE as soon as it is
            # decoded, all nfb*nch PSUM banks accumulate concurrently.
            # The last chunk is issued bank-by-bank so evacuation and
            # output DMA overlap the remaining j=7 matmuls.
            for j in range(7):
                for fb in range(nfb):
                    for ch in range(nch):
                        nc.tensor.matmul(
                            pss[(fb, ch)][:],
                            ws[j][:, fb * P : (fb + 1) * P],
                            xts[j][:, ch * MM_N : (ch + 1) * MM_N],
                            start=(j == 0 and (fb, ch) not in FILL_BANKS),
                            stop=False,
                        )
            # evacuation alternates DVE/ACT (GPSIMD cannot read PSUM) and
            # folds the bias in for free: DVE as tensor_scalar add with a
            # per-partition fp32 bias column, ACT as Identity with the AP
            # bias operand.  Banks pair into double-wide tiles; each pair
            # ships as ONE DMA, fb0 pairs on the gpsimd SWDGE queue and
            # fb1 pairs on SP/HWDGE so the issue pipes overlap.
            # first three bank-pairs ship as pair DMAs (gp/SP/gp), the last
            # two banks as singles on SP so the final transfer is short
            plan = [((0, 0), (0, 1), nc.gpsimd), ((0, 2), (0, 3), nc.sync),
                    ((1, 0), (1, 1), nc.gpsimd), ((1, 2), None, nc.sync),
                    ((1, 3), None, nc.sync)]
            k = 0
            evac_tiles = {}
            for banks in plan:
                b0, b1, deng = banks
                width = 2 * MM_N if b1 is not None else MM_N
                obw = op_.tile([P, width], BF16, tag="obw", name=f"obw{k}")
                for half, bk in enumerate((b0, b1)):
                    if bk is None:
                        continue
                    fb, ch = bk
                    bcol = biasc[:, fb : fb + 1]
                    nc.tensor.matmul(
                        pss[bk][:],
                        ws[7][:, fb * P : (fb + 1) * P],
                        xts[7][:, ch * MM_N : (ch + 1) * MM_N],
                        start=False,
                        stop=True,
                    )
                    dst = obw[:, half * MM_N : (half + 1) * MM_N]
                    if k % 2 == 0:
                        nc.vector.tensor_scalar(
                            dst, pss[bk][:], bcol, None, OP.add
                        )
                    else:
                        nc.scalar.activation(
                            dst, pss[bk][:], ACT_F.Identity, bias=bcol,
                        )
                    k += 1
                fb0_, ch0_ = b0
                deng.dma_start(
                    out_d[fb0_][:, ch0_ * MM_N : ch0_ * MM_N + width],
                    obw[:],
                )
    nc.compile()
    return nc


_I_PERM = 8 * (np.arange(NX) % G) + np.arange(NX) // G  # i' -> i


def host_prep(x, binary, scale, bias):
    """Layout-only sharding (plus x's fp16 compute-precision cast).
    Returns in_maps for cores 0..7 (pt = c//PF, pf = c%PF)."""
    x2 = np.ascontiguousarray(x.reshape(N_TOK, NX).T)[_I_PERM]  # (NX, N)
    x2 = x2.astype(BF16NP)  # compute dtype
    binary16 = binary.astype(np.int16)  # lossless: codes are 0..255
    in_maps = []
    for c in range(8):
        pt, pf = c // PF, c % PF
        f0 = pf * NFS
        xs = np.ascontiguousarray(x2[:, pt * TOK : (pt + 1) * TOK]).reshape(
            8, P, TOK
        )
        cs = np.ascontiguousarray(
            binary16[:, :, f0 : f0 + NFS].transpose(1, 0, 2)
        ).reshape(P, 8 * NFS)
        ss = np.ascontiguousarray(
            np.broadcast_to(
                (-scale[:, f0 : f0 + NFS].astype(BF16NP)).reshape(1, 8 * NFS),
                (P, 8 * NFS),
            )
        )
        bs = np.ascontiguousarray(
            bias[f0 : f0 + NFS].astype(np.float32).reshape(NFS // P, P).T
        )
        in_maps.append({"xt": xs, "codes": cs, "scales": ss, "biasc": bs})
    return in_maps


def host_assemble(results):
    """results[c]["out"]: (NFB, 128, TOK) -> full (B, T, NF)."""
    outT = np.empty((NF, N_TOK), dtype=np.float32)
    for c in range(8):
        pt, pf = c // PF, c % PF
        o = np.asarray(results[c]["out"], dtype=np.float32).reshape(NFS, TOK)
        outT[pf * NFS : (pf + 1) * NFS, pt * TOK : (pt + 1) * TOK] = o
    return np.ascontiguousarray(outT.T).reshape(B, T, NF)


_NC_CACHE = {}


def _get_nc():
    if "nc" not in _NC_CACHE:
        nc = bacc.Bacc(None, target_bir_lowering=False)
        build_graph(nc)
        _NC_CACHE["nc"] = nc
    return _NC_CACHE["nc"]


def kernel(**inputs):
    from concourse.bass_utils import run_bass_kernel_spmd

    inputs = {k: np.asarray(v) for k, v in inputs.items()}
    in_maps = host_prep(
        inputs["x"], inputs["binary"], inputs["scale"], inputs["bias"]
    )
    res = run_bass_kernel_spmd(_get_nc(), in_maps, core_ids=list(range(8)))
    return host_assemble(res.results)


# revision 21
# speedup vs baseline: 1.0328x; 1.0142x over previous
"""BQuantConv1d Trainium2 kernel.

Math: the reference's per-token LUT + gather is algebraically a matmul:
  out[n, f] = sum_i x[n, i] * W[i, f] + bias[f]
  W[8g+j, f] = sum_b scale[b, f] * (2*bit_{7-j}(binary[b, g, f]) - 1)

Sharding: 2 token-groups x 4 f-groups over 8 cores, no collectives
(host slices inputs / concatenates outputs; layout-only host work).
Contraction order is permuted to i' = j*128 + g (host permutes xT rows to
match) so each decoded weight chunk j lands on contiguous partitions.

Per core:
  - decode W'(1024, 256) from int16 codes with a sign-bit trick:
    W element = +-scale[b, f] exactly, built by XORing the fp16 scale's
    sign bit (scales arrive sign-pre-flipped) with the masked quant bit
    (c << (8+j)) & 0x8000, as int32 SWAR on DVE (bitvec ops are DVE-only
    and 32-bit-only on walrus); the 8-way b-reduction is an fp16 add
    tree with the first level (h1) on DVE and the h2/w levels offloaded
    to the otherwise-idle GPSIMD engine.  Chunks 0/1 run TS/xor in
    b-halves gated on half-sized cd/sc DMAs; chunk 7's whole tree runs
    on DVE split by f-half at every level so the fb=0 matmuls start
    while fb=1 is still reducing.
  - outT[f_shard, n_shard] = W'.T @ xT on the PE in fp16, accumulating
    the 8 contraction chunks across 8 concurrent PSUM banks (f32).
  - The PE p-state ramp (cost model: ~7us of continuous execution
    before full clock) is paid with low-priority zero-matmul fillers
    that keep the PE busy from ~1us instead of bias-seed matmuls; the
    filler bank's real accumulation chain is dep-deferred behind them
    and doubles as tail work.  The bias rides the PSUM evacuation for
    free (DVE tensor_scalar add with a per-partition fp32 bias column /
    ACT Identity activation with an AP bias operand).
  - PSUM pairs evacuated as fp16 (copies alternating DVE/ACT) into
    double-wide tiles; each ch-pair ships as ONE partition-major DMA,
    fb0 pairs on the gpsimd SWDGE queue, fb1 pairs on SP/HWDGE, so the
    two issue pipes overlap in the tail.
"""

import numpy as np

try:
    import concourse.bass as bass  # noqa: F401
except ImportError:
    import sys

    sys.path.insert(0, "/opt/trn_rl_repo")
    import concourse.bass as bass  # noqa: F401

import concourse.bacc as bacc
import concourse.mybir as mybir
import concourse.tile as tile

B, T, NX, NF = 2, 2048, 1024, 1024
N_TOK = B * T
BITS = 8
G = NX // 8  # 128 code groups
PT, PF = 2, 4  # token-parallel x feature-parallel
TOK = N_TOK // PT  # tokens per core
NFS = NF // PF  # output features per core
P = 128
MM_N = 512  # moving free dim per matmul
N_FILL = 24  # PE warm-up/pacing zero-matmuls (tuned against TimelineSim)

AX = mybir.AxisListType
OP = mybir.AluOpType
F32 = mybir.dt.float32
BF16 = mybir.dt.float16  # compute dtype (fp16: same SWAR, more mantissa)
I16 = mybir.dt.int16
I32 = mybir.dt.int32
ACT_F = mybir.ActivationFunctionType
BF16NP = np.float16

MSK = -2147450880  # 0x80008000 as int32


def build_graph(nc, tok=TOK, nfs=NFS):
    nfb = nfs // P  # f blocks of 128 (2)
    nch = tok // MM_N  # moving chunks (4)
    xt_d = nc.dram_tensor("xt", (8, P, tok), BF16, kind="ExternalInput")
    cd_d = nc.dram_tensor("codes", (P, 8 * nfs), I16, kind="ExternalInput")
    sc_d = nc.dram_tensor("scales", (P, 8 * nfs), BF16, kind="ExternalInput")
    bi_d = nc.dram_tensor("biasc", (P, nfb), F32, kind="ExternalInput")
    out_d = nc.dram_tensor("out", (nfb, P, tok), BF16, kind="ExternalOutput")
    # fb0 banks host the warm-up fillers; their real chains get dep-deferred
    # behind their fillers and become gap-fill work mid-stream
    FILL_BANKS = [(0, ch) for ch in range(nch)]

    with tile.TileContext(nc) as tc:
        with (
            tc.tile_pool(name="xp", bufs=8) as xp,
            tc.tile_pool(name="cp", bufs=8) as cp,
            tc.tile_pool(name="wp", bufs=8) as wp,
            tc.tile_pool(name="qp", bufs=6) as qp,
            tc.tile_pool(name="cst", bufs=1) as cst,
            tc.tile_pool(name="op", bufs=8) as op_,
            tc.tile_pool(name="pp", bufs=8, space="PSUM") as pp,
        ):
            # --- loads; codes/scales first, halves interleaved: chunk 0/1's
            # decode runs in b-halves gated on each half-DMA ---
            H = 4 * nfs
            cd = cp.tile([P, 8 * nfs], I16, tag="cd")
            sc_bc = cst.tile([P, 8 * nfs], BF16, tag="sc_bc")
            nc.sync.dma_start(cd[:, :H], cd_d[:, :H])
            nc.sync.dma_start(sc_bc[:, :H], sc_d[:, :H])
            nc.sync.dma_start(cd[:, H:], cd_d[:, H:])
            nc.sync.dma_start(sc_bc[:, H:], sc_d[:, H:])
            biasc = cst.tile([P, nfb], F32, tag="biasc")
            nc.sync.dma_start(biasc[:], bi_d[:])
            zero_m = cst.tile([P, MM_N], BF16, tag="zero_m")
            nc.gpsimd.memset(zero_m[:], 0.0)
            xts = []
            for j in range(8):
                xt = xp.tile([P, tok], BF16, tag="xt")
                nc.sync.dma_start(xt[:], xt_d[j])
                xts.append(xt)

            # PSUM banks.  FILLB is reset by the first zero-filler; every
            # other bank starts accumulation at its j=0 matmul.
            pss = {}
            for fb in range(nfb):
                for ch in range(nch):
                    ps = pp.tile([P, MM_N], F32, tag="ps", name=f"ps{fb}_{ch}")
                    pss[(fb, ch)] = ps

            # --- PE warm-up fillers: zero-weight matmuls spread across the
            # fb0 banks.  Low priority: the scheduler runs them only when
            # no real matmul is ready; they pay the cost model's ~10us PE
            # p-state ramp starting at ~0.9us, while the PE would
            # otherwise idle waiting for the first decoded W chunk.
            with tc.high_priority(offset=-(10**6)):
                for i in range(N_FILL):
                    bank = FILL_BANKS[i % len(FILL_BANKS)]
                    nc.tensor.matmul(
                        pss[bank][:], zero_m[:, :P], zero_m[:],
                        start=(i < len(FILL_BANKS)), stop=False,
                    )

            # --- decode W chunks ---
            # Sign-bit trick: masked quant bit (inverted) XORed onto the
            # fp16 scale's sign gives +-scale exactly.  Bitvec ops are
            # DVE-only and 32-bit-only on walrus, so they run as int32 SWAR
            # over int16-lane pairs: a left shift by 8+j sources each
            # lane's bit 15 from within the same lane, and the 0x80008000
            # mask keeps only the two sign bits.  The bit inversion is
            # folded into a one-time sign-flip of the scale tile:
            #   ((c << (8+j)) & M) ^ (sc ^ M)  ==  ((~c << (8+j)) & M) ^ sc
            def emit_ts(j):
                sg = qp.tile([P, 8 * nfs], I16, tag="sg", name=f"sg{j}")
                nc.vector.tensor_scalar(
                    sg[:].bitcast(I32), cd[:].bitcast(I32), 8 + j, MSK,
                    OP.logical_shift_left, OP.bitwise_and,
                )
                return sg

            def emit_xor(j, sg):
                wsg = qp.tile([P, 8 * nfs], I16, tag="wsg", name=f"wsg{j}")
                nc.vector.tensor_tensor(
                    wsg[:].bitcast(I32), sg[:].bitcast(I32),
                    sc_bc[:].bitcast(I32), OP.bitwise_xor,
                )
                return wsg

            def emit_h1(j, wsg):
                wv = wsg[:].bitcast(BF16)
                h1 = qp.tile([P, 4 * nfs], BF16, tag="h1", name=f"h1_{j}")
                nc.vector.tensor_tensor(
                    h1[:], wv[:, : 4 * nfs], wv[:, 4 * nfs :], OP.add
                )
                return h1

            def tree_tail(j, h1, teng):
                h2 = qp.tile([P, 2 * nfs], BF16, tag="h2", name=f"h2_{j}")
                teng.tensor_tensor(
                    h2[:], h1[:, : 2 * nfs], h1[:, 2 * nfs :], OP.add
                )
                w = wp.tile([P, nfs], BF16, tag="w", name=f"w{j}")
                teng.tensor_tensor(w[:], h2[:, :nfs], h2[:, nfs:], OP.add)
                return w

            # b-reduction fp16 add tree: h1 on DVE; h2/w on GPSIMD for
            # chunks 0-6.  The next chunk's TS is emitted BETWEEN xor_j and
            # h1_j: it is always ready, so it fills the write-ack window
            # after xor_j and the scheduler then runs h1_j instead of
            # parking the next 1.1us xor in front of it.
            ws = {}
            # chunks 0/1 in b-halves so each half starts on its half-DMA
            # (fills the DVE while the second cd/sc halves are in flight)
            sgs, wsgs = {}, {}
            for j in (0, 1):
                sg = qp.tile([P, 8 * nfs], I16, tag="sg", name=f"sg{j}")
                wsg = qp.tile([P, 8 * nfs], I16, tag="wsg", name=f"wsg{j}")
                for half in range(2):
                    lo, hi = half * H, (half + 1) * H
                    nc.vector.tensor_scalar(
                        sg[:, lo:hi].bitcast(I32), cd[:, lo:hi].bitcast(I32),
                        8 + j, MSK, OP.logical_shift_left, OP.bitwise_and,
                    )
                    nc.vector.tensor_tensor(
                        wsg[:, lo:hi].bitcast(I32),
                        sg[:, lo:hi].bitcast(I32),
                        sc_bc[:, lo:hi].bitcast(I32), OP.bitwise_xor,
                    )
                sgs[j], wsgs[j] = sg, wsg
            for j in range(1, 7):
                if j > 1:
                    sgs[j] = emit_ts(j)
                h1 = emit_h1(j - 1, wsgs[j - 1])
                ws[j - 1] = tree_tail(j - 1, h1, nc.gpsimd)
                if j > 1:
                    wsgs[j] = emit_xor(j, sgs[j])

            # --- endgame: chunks 6 and 7 reduce on DVE, f-half-split at
            # every tree level.  Chunk 6's W must land ~2us before chunk
            # 7's: each PSUM bank's j6->j7 accumulation is order-chained,
            # so a late W6 serializes 16 matmuls into the tail.  TS7 slots
            # between tree levels to fill the same-engine ack windows. ---
            def split_tree(j, wsg, w_tile, pieces):
                wv = wsg[:].bitcast(BF16).rearrange("p (b f) -> p b f", b=8)
                for half in range(2):
                    f0, f1 = half * P, (half + 1) * P
                    h1h = qp.tile([P, 4, P], BF16, tag="h1h",
                                  name=f"h1h{j}_{half}")
                    nc.vector.tensor_tensor(
                        h1h[:], wv[:, :4, f0:f1], wv[:, 4:, f0:f1], OP.add
                    )
                    h2h = qp.tile([P, 2, P], BF16, tag="h2h",
                                  name=f"h2h{j}_{half}")
                    nc.vector.tensor_tensor(h2h[:], h1h[:, :2], h1h[:, 2:],
                                            OP.add)
                    nc.vector.tensor_tensor(
                        w_tile[:, f0:f1], h2h[:, 0], h2h[:, 1], OP.add
                    )
                    if half == 0 and pieces:
                        pieces.pop()()

            w6 = wp.tile([P, nfs], BF16, tag="w", name="w6")
            w7 = wp.tile([P, nfs], BF16, tag="w", name="w7")
            split_tree(6, wsgs[6], w6, [lambda: sgs.__setitem__(7, emit_ts(7))])
            ws[6] = w6
            wsgs[7] = emit_xor(7, sgs[7])
            split_tree(7, wsgs[7], w7, [])
            ws[7] = w7

            # --- matmul: outT[f, n] = sum_j W_j.T @ xT_j (+bias at evac) ---
            # j outermost: each W chunk feeds the PE as soon as it is
            # decoded, all nfb*nch PSUM banks accumulate concurrently.
            # The last chunk is issued bank-by-bank so evacuation and
            # output DMA overlap the remaining j=7 matmuls.
            for j in range(7):
                for fb in range(nfb):
                    for ch in range(nch):
                        nc.tensor.matmul(
                            pss[(fb, ch)][:],
                            ws[j][:, fb * P : (fb + 1) * P],
                            xts[j][:, ch * MM_N : (ch + 1) * MM_N],
                            start=(j == 0 and (fb, ch) not in FILL_BANKS),
                            stop=False,
                        )
            # evacuation alternates DVE/ACT (GPSIMD cannot read PSUM) and
            # folds the bias in for free: DVE as tensor_scalar add with a
            # per-partition fp32 bias column, ACT as Identity with the AP
            # bias operand.  Banks pair into double-wide tiles; each pair
            # ships as ONE DMA, fb0 pairs on the gpsimd SWDGE queue and
            # fb1 pairs on SP/HWDGE so the issue pipes overlap.
            # first two bank-pairs ship on the gpsimd SWDGE queue (its 1us
            # desc-prep hides while later banks still matmul), the third on
            # SP, and the last two banks as singles on SP.  Banks 0-5 evac
            # with one op alternating DVE/ACT; the last two banks split
            # their evac across BOTH engines by f-half so the final DMA's
            # gate drops by ~0.25us.
            plan = [((0, 0), (0, 1), nc.gpsimd), ((0, 2), (0, 3), nc.gpsimd),
                    ((1, 0), (1, 1), nc.sync), ((1, 2), None, nc.sync),
                    ((1, 3), None, nc.sync)]
            k = 0
            for banks in plan:
                b0, b1, deng = banks
                width = 2 * MM_N if b1 is not None else MM_N
                obw = op_.tile([P, width], BF16, tag="obw", name=f"obw{k}")
                for half, bk in enumerate((b0, b1)):
                    if bk is None:
                        continue
                    fb, ch = bk
                    bcol = biasc[:, fb : fb + 1]
                    nc.tensor.matmul(
                        pss[bk][:],
                        ws[7][:, fb * P : (fb + 1) * P],
                        xts[7][:, ch * MM_N : (ch + 1) * MM_N],
                        start=False,
                        stop=True,
                    )
                    dst = obw[:, half * MM_N : (half + 1) * MM_N]
                    if b1 is None:
                        # split evac: DVE low half, ACT high half
                        nc.vector.tensor_scalar(
                            dst[:, : MM_N // 2], pss[bk][:, : MM_N // 2],
                            bcol, None, OP.add,
                        )
                        nc.scalar.activation(
                            dst[:, MM_N // 2 :], pss[bk][:, MM_N // 2 :],
                            ACT_F.Identity, bias=bcol,
                        )
                    elif k % 2 == 0:
                        nc.vector.tensor_scalar(
                            dst, pss[bk][:], bcol, None, OP.add
                        )
                    else:
                        nc.scalar.activation(
                            dst, pss[bk][:], ACT_F.Identity, bias=bcol,
                        )
                    k += 1
                fb0_, ch0_ = b0
                deng.dma_start(
                    out_d[fb0_][:, ch0_ * MM_N : ch0_ * MM_N + width],
                    obw[:],
                )
    nc.compile()
    return nc


_I_PERM = 8 * (np.arange(NX) % G) + np.arange(NX) // G  # i' -> i


def host_prep(x, binary, scale, bias):
    """Layout-only sharding (plus x's fp16 compute-precision cast).
    Returns in_maps for cores 0..7 (pt = c//PF, pf = c%PF)."""
    x2 = np.ascontiguousarray(x.reshape(N_TOK, NX).T)[_I_PERM]  # (NX, N)
    x2 = x2.astype(BF16NP)  # compute dtype
    binary16 = binary.astype(np.int16)  # lossless: codes are 0..255
    in_maps = []
    for c in range(8):
        pt, pf = c // PF, c % PF
        f0 = pf * NFS
        xs = np.ascontiguousarray(x2[:, pt * TOK : (pt + 1) * TOK]).reshape(
            8, P, TOK
        )
        cs = np.ascontiguousarray(
            binary16[:, :, f0 : f0 + NFS].transpose(1, 0, 2)
        ).reshape(P, 8 * NFS)
        ss = np.ascontiguousarray(
            np.broadcast_to(
                (-scale[:, f0 : f0 + NFS].astype(BF16NP)).reshape(1, 8 * NFS),
                (P, 8 * NFS),
            )
        )
        bs = np.ascontiguousarray(
            bias[f0 : f0 + NFS].astype(np.float32).reshape(NFS // P, P).T
        )
        in_maps.append({"xt": xs, "codes": cs, "scales": ss, "biasc": bs})
    return in_maps


def host_assemble(results):
    """results[c]["out"]: (NFB, 128, TOK) -> full (B, T, NF)."""
    outT = np.empty((NF, N_TOK), dtype=np.float32)
    for c in range(8):
        pt, pf = c // PF, c % PF
        o = np.asarray(results[c]["out"], dtype=np.float32).reshape(NFS, TOK)
        outT[pf * NFS : (pf + 1) * NFS, pt * TOK : (pt + 1) * TOK] = o
    return np.ascontiguousarray(outT.T).reshape(B, T, NF)


_NC_CACHE = {}


def _get_nc():
    if "nc" not in _NC_CACHE:
        nc = bacc.Bacc(None, target_bir_lowering=False)
        build_graph(nc)
        _NC_CACHE["nc"] = nc
    return _NC_CACHE["nc"]


def kernel(**inputs):
    from concourse.bass_utils import run_bass_kernel_spmd

    inputs = {k: np.asarray(v) for k, v in inputs.items()}
    in_maps = host_prep(
        inputs["x"], inputs["binary"], inputs["scale"], inputs["bias"]
    )
    res = run_bass_kernel_spmd(_get_nc(), in_maps, core_ids=list(range(8)))
    return host_assemble(res.results)
